# revision 1
# baseline (speedup 1.0000x reference)
"""CRF loss kernel for Trainium2 (8 NeuronCores, data-parallel over batch).

Math (faithful to the reference):
  loss = (forscore - tg_energy) / B
  tg_energy = B*trans[0,START] + sum_bt scores[b,t,0] + sum_bt trans[0, gold[b,t]]
    (the reference's torch.gather-on-flattened-(L*L) quirk reduces to row 0)
  forscore = sum_b fs_T[b, END], where fs is the standard CRF forward recurrence
    fs_{t+1}[j] = logsumexp_i(fs_t[i] + scores[t,i] + trans[i,j]), fs_0 = trans[START,:]

Device algorithm, linear space with E = exp(trans) (bf16 matmuls, f32 PSUM):
  forward half  (t = 0..T/2-1):    w_{t+1} = E^T (w_t  * s_t),  w_0 = exp(trans[START,:])
  backward half (t = T-1..T/2):    r_t     = s_t * (E r_{t+1}), r_T = e_END  (adjoint)
  e_END^T w_T = r_{T/2}^T w_{T/2}  -> one dot product at the junction.
The two 256-step chains are independent, so each one's (DVE mul -> PE matmul ->
sem) latency hides inside the other's gaps: ~256 dependent rounds, not 512.
s_t = exp(scores_t - delta) is produced by the scalar engine (Exp only — no
activation-table thrashing).

Magnitude control: at each chunk boundary (CHS schedule) each chain measures
z = 1^T y via a tiny matmul; 1/z (vector reciprocal, bf16) is broadcast across
partitions by a K=1 matmul and applied as a one-step multiplicative jolt to
that chain's exp'd-score stream two chunks later. The f32 z values stream to
DRAM and the host adds back sum(log z) over the applied corrections:
  fs_T[b] = log(dot[b]) + T*delta + sum_c log zA[c,b] + sum_c log zB[c,b].

Per-core layout: tags on partitions (48), local batch (8) on the free dim.
mask is all ones per the problem spec (fill: ones), so the mask gating
(where(mask, nxt, fs)) is the identity and is not materialized on device.
"""

import numpy as np

B, T, L = 64, 512, 48
START, PAD, END = 46, 45, 47
NCORES = 8
BL = B // NCORES          # 8 batch elements per core
H = T // 2                # steps per chain
CHS = [8, 24] + [32] * 7  # renorm chunk sizes (small first chunk -> the first
                          # DMA+exp gating the chain start is tiny)
SOFF = [sum(CHS[:i]) for i in range(len(CHS))]
NCH2 = len(CHS)           # chunks per chain
LAG = 2                   # feedback delay (chunks) for the 1/z correction
DELTA = 5.0               # static per-step log shift folded into exp(scores)

_NC_CACHE = {}


def build_nc():
    import concourse.bacc as bacc
    import concourse.mybir as mybir
    import concourse.tile as tile

    f32 = mybir.dt.float32
    bf16 = mybir.dt.bfloat16
    AF = mybir.ActivationFunctionType
    AL = mybir.AluOpType
    AX = mybir.AxisListType

    nc = bacc.Bacc("TRN2", target_bir_lowering=False, debug=False)

    s_dram = nc.dram_tensor("s_tr", [L, T * BL], f32, kind="ExternalInput")
    trans_d = nc.dram_tensor("trans", [L, L], f32, kind="ExternalInput")
    transT_d = nc.dram_tensor("transT", [L, L], f32, kind="ExternalInput")
    goldf_d = nc.dram_tensor("goldf", [128, 32], f32, kind="ExternalInput")
    sc0_d = nc.dram_tensor("sc0", [128, 32], f32, kind="ExternalInput")
    iota_d = nc.dram_tensor("iotaf", [128, L], f32, kind="ExternalInput")

    # one output tensor: [ zA(128) | zB(128) | dot(8) | tg_gather, sc0_sum ]
    out_d = nc.dram_tensor(
        "out_all", [1, 2 * NCH2 * BL + BL + 2], f32, kind="ExternalOutput")

    with tile.TileContext(nc) as tc:
        with (
            tc.tile_pool(name="const", bufs=1) as cpool,
            tc.tile_pool(name="sraw", bufs=4) as rpool,
            tc.tile_pool(name="sexp", bufs=4) as epool,
            tc.tile_pool(name="yy", bufs=4) as ypool,
            tc.tile_pool(name="small", bufs=4) as smpool,
            tc.tile_pool(name="oh", bufs=2) as ohpool,
            tc.tile_pool(name="wps", bufs=2, space="PSUM") as wpool,
            tc.tile_pool(name="qps", bufs=2, space="PSUM") as qpool,
            tc.tile_pool(name="zps", bufs=2, space="PSUM") as zpool,
            tc.tile_pool(name="cntps", bufs=1, space="PSUM") as cntpool,
            tc.tile_pool(name="cbps", bufs=1, space="PSUM") as cbpool,
        ):
            # ---- startup-critical DMAs first (transT gates the chain-A init
            # scalar; raw chunks gate the first steps); other constants ride
            # the (otherwise idle) gpsimd DMA queue ----
            raw0A = rpool.tile([L, CHS[0] * BL], f32, tag="rawA")
            nc.sync.dma_start(raw0A[:], s_dram[:, 0:CHS[0] * BL])
            transT_sb = cpool.tile([L, L], f32)
            nc.sync.dma_start(transT_sb[:], transT_d[:])

            trans_sb = cpool.tile([L, L], f32)
            nc.gpsimd.dma_start(trans_sb[:], trans_d[:])
            raw0B = rpool.tile([L, CHS[0] * BL], f32, tag="rawB")
            nc.gpsimd.dma_start(raw0B[:], s_dram[:, (T - CHS[0]) * BL:T * BL])

            zero48 = cpool.tile([L, 1], f32)
            nc.vector.memset(zero48[:], 0.0)
            negd48 = cpool.tile([L, 1], f32)
            nc.vector.memset(negd48[:], -DELTA)

            # prefetch the Exp activation table while the DMAs run
            warm_act = cpool.tile([L, 1], f32)
            nc.scalar.activation(warm_act[:], zero48[:], AF.Exp, bias=zero48[:])

            # only two f32 columns are needed for the chain inits:
            # exp(trans[START,:]) = exp(transT[:,START]) and exp(trans[:,END]);
            # (48,1) exps keep the startup ACT train short
            ETcol = cpool.tile([L, 1], f32)
            nc.scalar.activation(
                ETcol[:], transT_sb[:, START:START + 1], AF.Exp, bias=zero48[:])
            E_bf = cpool.tile([L, L], bf16)
            nc.scalar.activation(E_bf[:], trans_sb[:], AF.Exp, bias=zero48[:])
            Ecol = cpool.tile([L, 1], f32)
            nc.scalar.activation(
                Ecol[:], trans_sb[:, END:END + 1], AF.Exp, bias=zero48[:])
            ET_bf = cpool.tile([L, L], bf16)
            nc.scalar.activation(ET_bf[:], transT_sb[:], AF.Exp, bias=zero48[:])

            ones48b = cpool.tile([L, 1], bf16)
            nc.vector.memset(ones48b[:], 1.0)
            ones48f = cpool.tile([L, 1], f32)
            nc.vector.memset(ones48f[:], 1.0)
            ones1x48 = cpool.tile([1, L], bf16)
            nc.vector.memset(ones1x48[:], 1.0)
            ones128b = cpool.tile([128, 1], bf16)
            nc.vector.memset(ones128b[:], 1.0)
            ones128f = cpool.tile([128, 1], f32)
            nc.vector.memset(ones128f[:], 1.0)
            zbuf = cpool.tile([1, 2 * NCH2 * BL + BL + 2], f32)
            ZD = 2 * NCH2 * BL          # dot offset in zbuf
            ZS = ZD + BL                # (tg_gather, sc0_sum) offset

            # gold-histogram inputs (consumed from chunk 0 onward, off-path)
            iota_sb = cpool.tile([128, L], f32)
            nc.gpsimd.dma_start(iota_sb[:], iota_d[:])
            goldf_sb = cpool.tile([128, 32], f32)
            nc.gpsimd.dma_start(goldf_sb[:], goldf_d[:])
            sc0_sb = cpool.tile([128, 32], f32)
            nc.gpsimd.dma_start(sc0_sb[:], sc0_d[:])
            cnt_ps = cntpool.tile([L, 1], f32)

            # ---- twin 256-step chains, interleaved ----
            cbA, cbB = {}, {}
            w_prev = None     # chain A state (PSUM)
            q_prev = None     # chain B state (PSUM)
            yA = yB = None
            pending_fb = []   # feedback ops deferred into the next chunk so
                              # the in-order PE queue isn't head-blocked on DVE
            pending_z = []    # z-measure matmuls, deferred one round likewise
            hist_cc = 0
            for c in range(NCH2):
                K = CHS[c]
                s0, s1 = SOFF[c], SOFF[c] + K
                if c == 0:
                    rawA, rawB = raw0A, raw0B
                else:
                    rawA = rpool.tile([L, K * BL], f32, tag="rawA")
                    nc.sync.dma_start(rawA[:], s_dram[:, s0 * BL:s1 * BL])
                    rawB = rpool.tile([L, K * BL], f32, tag="rawB")
                    nc.sync.dma_start(
                        rawB[:], s_dram[:, (T - s1) * BL:(T - s0) * BL])
                seA = epool.tile([L, K, BL], f32, tag="seA")
                nc.scalar.activation(
                    seA[:].rearrange("p a b -> p (a b)"), rawA[:], AF.Exp,
                    bias=negd48[:])
                seB = epool.tile([L, K, BL], f32, tag="seB")
                nc.scalar.activation(
                    seB[:].rearrange("p a b -> p (a b)"), rawB[:], AF.Exp,
                    bias=negd48[:])

                if c in cbA:
                    s0cA = smpool.tile([L, BL], f32, tag="s0cA")
                    nc.vector.tensor_tensor(
                        s0cA[:], seA[:, 0, :], cbA.pop(c)[:, 0:BL], AL.mult)
                else:
                    s0cA = None
                if c in cbB:
                    s0cB = smpool.tile([L, BL], f32, tag="s0cB")
                    nc.vector.tensor_tensor(
                        s0cB[:], seB[:, K - 1, :], cbB.pop(c)[:, BL:2 * BL],
                        AL.mult)
                else:
                    s0cB = None

                for k in range(K):
                    if k == 1 and pending_z:
                        for zb in pending_z:
                            zb()
                        pending_z = []
                    if k == 6 and pending_fb:
                        for fb in pending_fb:
                            fb()
                        pending_fb = []
                    if c >= 1 and k % 6 == 3 and hist_cc < 32:
                        # fold gold-histogram pieces into the chunks'
                        # DVE/PE slack instead of a serial tail
                        cc = hist_cc
                        hist_cc += 1
                        oh = ohpool.tile([128, L], bf16, tag="oh")
                        nc.vector.tensor_scalar(
                            oh[:], iota_sb[:], goldf_sb[:, cc:cc + 1], None,
                            AL.is_equal)
                        nc.tensor.matmul(
                            cnt_ps[:], oh[:], ones128b[:],
                            start=(cc == 0), stop=(cc == 31))
                        if cc == 31:
                            # tg epilogue, inside the loop so it overlaps the
                            # final chunk instead of serializing after it
                            cnt_sb = smpool.tile([L, 1], f32, tag="cnt")
                            nc.vector.tensor_copy(cnt_sb[:], cnt_ps[:])
                            tgg_ps = zpool.tile([1, 1], f32, tag="z")
                            nc.tensor.matmul(
                                tgg_ps[:], cnt_sb[:], transT_sb[:, 0:1],
                                start=True, stop=True)
                            nc.vector.tensor_copy(
                                zbuf[:, ZS:ZS + 1], tgg_ps[:])
                            red = smpool.tile([128, 1], f32, tag="red")
                            nc.vector.reduce_sum(red[:], sc0_sb[:], axis=AX.X)
                            sc_ps = zpool.tile([1, 1], f32, tag="z")
                            nc.tensor.matmul(
                                sc_ps[:], red[:], ones128f[:],
                                start=True, stop=True)
                            nc.vector.tensor_copy(
                                zbuf[:, ZS + 1:ZS + 2], sc_ps[:])
                    # chain A, step = s0 + k (ascending t)
                    sA = s0cA[:] if (k == 0 and s0cA is not None) else seA[:, k, :]
                    yA = ypool.tile([L, BL], bf16, tag="yA")
                    if c == 0 and k == 0:
                        nc.vector.tensor_scalar_mul(
                            yA[:], sA, ETcol[:])
                    else:
                        nc.vector.tensor_tensor(yA[:], w_prev[:], sA, AL.mult)
                    w_prev = wpool.tile([L, BL], f32, tag="w")
                    nc.tensor.matmul(
                        w_prev[:], E_bf[:], yA[:], start=True, stop=True)

                    # chain B, t = T-1 - (c*K + k) (descending); kk indexes seB
                    kk = K - 1 - k
                    last_b = (c == NCH2 - 1 and k == K - 1)
                    sB = s0cB[:] if (k == 0 and s0cB is not None) else seB[:, kk, :]
                    yB = ypool.tile([L, BL], f32 if last_b else bf16, tag="yB")
                    if c == 0 and k == 0:
                        nc.vector.tensor_scalar_mul(
                            yB[:], sB, Ecol[:])
                    else:
                        nc.vector.tensor_tensor(yB[:], q_prev[:], sB, AL.mult)
                    if not last_b:      # r_{T/2} itself never enters a matmul
                        q_prev = qpool.tile([L, BL], f32, tag="q")
                        nc.tensor.matmul(
                            q_prev[:], ET_bf[:], yB[:], start=True, stop=True)

                # chunk-end magnitude measurement + delayed 1/z feedback;
                # both chains' broadcast factors share one PSUM tile (A|B).
                # The z matmuls + copies run one round into the next chunk and
                # the reciprocal + broadcast five rounds later, so neither the
                # PE nor the DVE queue head-blocks at the chunk seam.
                if c + LAG < NCH2:
                    cbt = cbpool.tile([L, 2 * BL], f32, tag="cb")
                else:
                    cbt = None
                last_chunk = (c == NCH2 - 1)
                for name, ytile, cbmap, zoff, cbsl in (
                        ("A", yA, cbA, c * BL, (0, BL)),
                        ("B", yB, cbB, (NCH2 + c) * BL, (BL, 2 * BL))):
                    def _zb(name=name, ytile=ytile, zoff=zoff, cbsl=cbsl,
                            cbt=cbt, cbmap=cbmap, c=c, last_chunk=last_chunk):
                        z_ps = zpool.tile([1, BL], f32, tag="z")
                        lhs1 = ones48f if (name == "B" and last_chunk) else ones48b
                        nc.tensor.matmul(
                            z_ps[:], lhs1[:], ytile[:], start=True, stop=True)
                        nc.vector.tensor_copy(zbuf[:, zoff:zoff + BL], z_ps[:])
                        if cbt is not None:
                            def _fb(z_ps=z_ps, cbt=cbt, cbsl=cbsl, name=name):
                                zr = smpool.tile([1, BL], bf16, tag="zr" + name)
                                # bf16 rounding of the 1/z factor only shifts
                                # which factor was applied; harmless (log z is
                                # re-added on the host from the f32 z_out)
                                with nc.allow_low_precision(
                                        reason="renorm factor"):
                                    nc.vector.reciprocal(zr[:], z_ps[:])
                                nc.tensor.matmul(
                                    cbt[:, cbsl[0]:cbsl[1]], ones1x48[:], zr[:],
                                    start=True, stop=True)
                            pending_fb.append(_fb)
                            cbmap[c + LAG] = cbt
                    if last_chunk:
                        _zb()
                    else:
                        pending_z.append(_zb)

            # junction dot product: e_END^T w_T = r_{T/2}^T w_{T/2}
            dprod = smpool.tile([L, BL], f32, tag="dprod")
            nc.vector.tensor_tensor(dprod[:], w_prev[:], yB[:], AL.mult)
            d_ps = zpool.tile([1, BL], f32, tag="z")
            nc.tensor.matmul(d_ps[:], ones48f[:], dprod[:], start=True, stop=True)
            nc.vector.tensor_copy(zbuf[:, ZD:ZD + BL], d_ps[:])
            nc.sync.dma_start(out_d[:], zbuf[:])

    nc.compile()
    return nc


def _get_nc():
    if "nc" not in _NC_CACHE:
        _NC_CACHE["nc"] = build_nc()
    return _NC_CACHE["nc"]


def make_in_maps(scores, gold_target, transitions):
    scores = np.asarray(scores, dtype=np.float32)
    gold = np.asarray(gold_target)
    trans = np.ascontiguousarray(np.asarray(transitions, dtype=np.float32))
    transT = np.ascontiguousarray(trans.T)
    iota = np.ascontiguousarray(
        np.broadcast_to(np.arange(L, dtype=np.float32)[None, :], (128, L)))
    in_maps = []
    for c in range(NCORES):
        sc = scores[c * BL:(c + 1) * BL]                     # (BL, T, L)
        s_tr = np.ascontiguousarray(sc.transpose(2, 1, 0)).reshape(L, T * BL)
        goldf = np.ascontiguousarray(
            gold[c * BL:(c + 1) * BL].astype(np.float32).reshape(128, 32))
        sc0 = np.ascontiguousarray(sc[:, :, 0].astype(np.float32).reshape(128, 32))
        in_maps.append({
            "s_tr": s_tr, "trans": trans, "transT": transT,
            "goldf": goldf, "sc0": sc0, "iotaf": iota,
        })
    return in_maps


def combine_outputs(results, transitions):
    trans = np.asarray(transitions, dtype=np.float64)
    forscore = 0.0
    tg_energy = 0.0
    nz = 2 * NCH2 * BL
    for c in range(NCORES):
        out = np.asarray(results[c]["out_all"], dtype=np.float64)[0]
        zv = out[:nz].reshape(2, NCH2, BL)
        dv = out[nz:nz + BL]
        tgg, sc0s = out[nz + BL], out[nz + BL + 1]
        fs_end = (np.log(dv) + DELTA * T
                  + np.log(zv[0, :NCH2 - LAG]).sum(axis=0)
                  + np.log(zv[1, :NCH2 - LAG]).sum(axis=0))
        forscore += fs_end.sum()
        tg_energy += tgg + sc0s + BL * trans[0, START]
    return np.float32((forscore - tg_energy) / B)


def kernel(scores, gold_target, mask, transitions):
    from concourse.bass_utils import run_bass_kernel_spmd

    nc = _get_nc()
    in_maps = make_in_maps(scores, gold_target, transitions)
    res = run_bass_kernel_spmd(nc, in_maps, list(range(NCORES)))
    return combine_outputs(res.results, transitions)

